# revision 22
# baseline (speedup 1.0000x reference)
"""MoE expert-combine kernel for Trainium2 (raw Bass, hand-scheduled), 8-core SPMD.

Problem: out[b,s,:] = sum_k expert_weights[b,s,k] * expert_outputs[expert_indices[b,s,k], b, s, :]
  B,S,H = 4,2048,1024 ; E=8 ; K=2  (hidden_states is unused by the reference)

Sharding: flatten tokens t = b*S+s (8192 total); each of the 8 cores owns a
contiguous block of 1024 tokens. Each core receives the expert-output stack
sliced to its tokens and downcast to bf16 ([E, 1024, H] viewed as a row table
[E*1024, H]) plus host-precomputed gather row indices and f32 gate weights.
The output is written bf16 (partition-major) and upcast/reordered to f32 on
the host; the combined quantization error is ~2.5e-3 rel, far inside the 2e-2
gate, and it halves the DMA traffic (12MB -> 6MB per core).

Device schedule, per 128-token chunk c (token = c*128 + p):
 - gather: 4 SWDGE dma_gather ops (mlp gpsimd library), one per chunk PAIR
   (512 rows of 2KB each), round-robin across the 4 SWDGE queues. One op is
   ~1.3us of Q7 descriptor writing, so 4 ops keep the Q7 off the critical
   path while the 4 rings transfer in parallel. Indices are int16 in the
   documented [16-partition wrap x replicated-across-cores] layout.
 - combine, split across two engines so neither is the bottleneck:
     Act:  acc[c%4] = w0 * g0      (Copy activation with per-partition scale)
     DVE:  ot[c] = (w1 * g1) + acc (scalar_tensor_tensor)
   acc is a 4-deep ring buffer; Act waits on sem_v for the anti-dependency
   before reusing a slot (standalone wait; the op's own wait slot is spent on
   the gather semaphore).
 - store: HWDGE writes chunk pairs as [128, 2048] bf16 to a partition-major
   DRAM layout ([P, NCHUNK*H]) so each store descriptor is a contiguous 4KB.
Hand-placed semaphores, at most one sync-wait per compute instruction (walrus
codegen limit), and no end-of-block drain/barrier (the sync engine's final
sem_st wait covers every data dependency; the NEFF's own per-engine completion
chain runs regardless).
"""

import sys
import numpy as np

for _p in ("/opt/trn_rl_repo", "/opt/pypackages"):
    if _p not in sys.path:
        sys.path.append(_p)

import ml_dtypes

from concourse import bass, mybir
from concourse.bass_utils import run_bass_kernel_spmd
from concourse.library_config import mlp as _mlp_lib

B, S, H = 4, 2048, 1024
E, K = 8, 2
N_CORES = 8
T = B * S              # 8192 tokens total
TC = T // N_CORES      # 1024 tokens per core
P = 128                # SBUF partitions
NCHUNK = TC // P       # 8 chunks of 128 tokens per core

N_GOPS = 4             # gather ops per core (2 chunks each)
CPG = NCHUNK // N_GOPS  # chunks per gather op
RPG = CPG * P * K       # gather rows per op (512)
# one op per SWDGE queue: transfers drain the queues in firing order, so
# chunk pairs complete in op order every ~3us and the combine pipelines behind
GOP_QUEUE = [0, 1, 2, 3]
N_SWDGE_QUEUES = 4
ACC_DEPTH = 4           # acc ring buffer depth

_f32 = mybir.dt.float32
_bf16 = mybir.dt.bfloat16
_i16 = mybir.dt.int16

_BF16 = ml_dtypes.bfloat16


def _build():
    nc = bass.Bass(
        target_bir_lowering=False,
        dynamic_dma_scratch_size=32768,
        num_swdge_queues=N_SWDGE_QUEUES,
    )

    # Preamble instructions exist already (emitted by Bass.__init__); snapshot
    # them so the strip below touches only these, never user instructions.
    _preamble_names = {
        ins.name for bb in nc.m.functions[0].blocks for ins in bb.instructions
    }

    table = nc.declare_dram_parameter("table", [E * TC, H], _bf16, isOutput=False)
    # gather indices, int16, dma_gather wrap layout: position i of gather op g
    # at [i % 16, g*(RPG//16) + i//16], replicated across the 8 core stripes
    gidx = nc.declare_dram_parameter(
        "gidx", [P, NCHUNK * K * P // 16], _i16, isOutput=False
    )
    wgt = nc.declare_dram_parameter("wgt", [P, NCHUNK * K], _f32, isOutput=False)
    # partition-major output: row p holds tokens (c*128+p) for c = 0..NCHUNK-1
    out = nc.declare_dram_parameter("out", [P, NCHUNK * H], _bf16, isOutput=True)

    with (
        nc.semaphore("sem_idx") as sem_idx,
        nc.semaphore("sem_prep") as sem_prep,
        nc.semaphore("sem_w") as sem_w,
        nc.semaphore("sem_acc") as sem_acc,
        nc.semaphore("sem_v") as sem_v,
        nc.semaphore("sem_st") as sem_st,
        nc.sbuf_tensor("gidx_t", [P, NCHUNK * K * P // 16], _i16) as gidx_t,
        nc.sbuf_tensor("warm_idx", [P, 8], _i16) as warm_idx,
        nc.sbuf_tensor("w_t", [P, NCHUNK * K], _f32) as w_t,
        nc.sbuf_tensor("g_t", [P, NCHUNK * K, H], _bf16) as g_t,
        nc.sbuf_tensor("ot_t", [P, NCHUNK * H], _bf16) as ot_t,
        nc.sbuf_tensor("acc_t", [P, ACC_DEPTH, H], _bf16) as acc_t,
    ):
        gather_sems = [nc.alloc_semaphore(f"sem_g{i}") for i in range(N_GOPS)]

        def sync_body(sync: bass.BassEngine):
            sync.dma_start(out=gidx_t[:], in_=gidx[:]).then_inc(sem_idx, 16)
            sync.dma_start(out=w_t[:], in_=wgt[:]).then_inc(sem_w, 16)
            for j in range(NCHUNK // 2):
                # chunk pair (2j, 2j+1) ready after DVE op 2j+2; the DRAM side
                # is contiguous per partition -> 4KB store descriptors
                sync.wait_ge(sem_v, 2 * j + 2)
                sync.dma_start(
                    out=out[:, 2 * j * H : (2 * j + 2) * H],
                    in_=ot_t[:, 2 * j * H : (2 * j + 2) * H],
                ).then_inc(sem_st, 16)
            # Final wait: keeps every sem update inside the program (safe for
            # re-execution). Costs nothing — the runtime teardown's per-engine
            # DRAINs wait for DMA-queue quiescence anyway.
            sync.wait_ge(sem_st, 16 * (NCHUNK // 2))

        def gpsimd_body(gpsimd: bass.BassGpSimd):
            # prepare+trigger per op: the self-triggered path only fires the
            # rings after the whole gpsimd instruction stream finishes its
            # descriptor writing (~10us); an explicit trigger right after each
            # prep starts that op's transfers immediately.
            gpsimd.load_library(_mlp_lib)
            # Warm-up: the FIRST dma_gather after a library load pays the Q7
            # gather-kernel init and waits out the library-load latency
            # (measured 1.2us best / 10us worst, directly on the critical
            # path). A dummy 128-row gather of table row 0 (zeroed indices,
            # landing in g_t chunk 0 which real op 0 overwrites later on the
            # same queue) absorbs all of that while the index DMA is still in
            # flight.
            gpsimd.memset(warm_idx[:], 0)
            gpsimd.dma_gather(
                g_t[:, 0:1, :],
                table[:],
                warm_idx[:],
                P,
                P,
                H,
                queue_num=0,
            ).then_inc(sem_prep, 16)
            gpsimd.wait_ge(sem_idx, 16)
            for g in range(N_GOPS):
                gpsimd.dma_gather(
                    g_t[:, g * CPG * K : (g + 1) * CPG * K, :],
                    table[:],
                    gidx_t[:, g * (RPG // 16) : (g + 1) * (RPG // 16)],
                    RPG,
                    RPG,
                    H,
                    queue_num=GOP_QUEUE[g],
                ).then_inc(gather_sems[g], 16)
                if g == N_GOPS - 2:
                    # Single pacing wait: self-triggered SWDGE doorbells only
                    # flush when the Q7 idles (or at stream end), so without
                    # this ALL desc-writing (~10us) serializes before ANY
                    # transfer. Idling here flushes ops 0..g at once — their
                    # ~9us of transfers overlap the last op's write. Only that
                    # one write pays the ~2x ring-contention rate (a full
                    # per-op pacing cascade measured slower: every write
                    # contended).
                    gpsimd.wait_ge(gather_sems[0], 16)

        def scalar_body(scalar: bass.BassEngine):
            # acc[c % ACC_DEPTH] = w0 * g0 on the Activation engine
            scalar.wait_ge(sem_w, 16)
            for c in range(NCHUNK):
                if c >= ACC_DEPTH:
                    # anti-dependency: DVE must have consumed acc slot c-4
                    scalar.wait_ge(sem_v, c - ACC_DEPTH + 1)
                m0 = c * K
                scalar.mul(
                    acc_t[:, c % ACC_DEPTH, :],
                    g_t[:, m0, :],
                    w_t[:, m0 : m0 + 1],
                )._wait_ge(gather_sems[c // CPG], 16).then_inc(sem_acc, 1)

        def vector_body(vector: bass.BassEngine):
            # ot[c] = (w1 * g1) + acc[c % ACC_DEPTH] ; the acc sem implies the
            # chunk's gather completed (Act waited on it), so one wait suffices
            vector.wait_ge(sem_w, 16)
            for c in range(NCHUNK):
                m1 = c * K + 1
                vector.scalar_tensor_tensor(
                    out=ot_t[:, c * H : (c + 1) * H],
                    in0=g_t[:, m1, :],
                    scalar=w_t[:, m1 : m1 + 1],
                    in1=acc_t[:, c % ACC_DEPTH, :],
                    op0=mybir.AluOpType.mult,
                    op1=mybir.AluOpType.add,
                )._wait_ge(sem_acc, c + 1).then_inc(sem_v, 1)

        # Emit every engine's stream directly into the entry basic block: no
        # per-engine body blocks means no branches, so the sequencers never
        # stall on an IRAM block fetch (~2.5us observed), and there is no
        # end-of-block drain/barrier either.
        sync_body(nc.sync)
        gpsimd_body(nc.gpsimd)
        scalar_body(nc.scalar)
        vector_body(nc.vector)

    # Strip the preamble's const-tile memsets and the post-init all-engine
    # barrier (~2.5us): this kernel never reads the const APs, and each
    # engine's register init precedes its user code in program order anyway.
    entry = nc.m.functions[0].blocks[0]
    drop = {
        ins.name
        for ins in entry.instructions
        if ins.name in _preamble_names
        and type(ins).__name__
        in ("InstMemset", "InstDrain", "InstEventSemaphore", "InstRegisterMove")
    }
    kept = [ins for ins in entry.instructions if ins.name not in drop]
    del entry.instructions[:]
    for ins in kept:
        entry.instructions.append(ins)

    # Lower InstISA pseudo-instructions (the mlp-library reload) to real ISA
    # bytes; raw walrus codegen rejects unlowered pseudos.
    mybir.codegen_inst_isa_subclasses(nc)

    nc.finalize()
    return nc


def _prepare_in_maps(expert_indices, expert_weights, expert_outputs):
    eo = np.asarray(expert_outputs, dtype=np.float32).reshape(E, T, H)
    eo16 = eo.astype(_BF16)
    flat_idx = np.asarray(expert_indices).reshape(T, K).astype(np.int32)
    flat_w = np.asarray(expert_weights, dtype=np.float32).reshape(T, K)
    t_local = np.arange(TC, dtype=np.int32)[:, None]
    in_maps = []
    for i in range(N_CORES):
        t0 = i * TC
        slab = np.ascontiguousarray(eo16[:, t0 : t0 + TC, :]).reshape(E * TC, H)
        li = flat_idx[t0 : t0 + TC] * TC + t_local  # [TC, K] row idx into slab
        # dma_gather layout: gather position i of the whole core is
        # i = m*128 + p with m = c*K + k (so g_t[p, m, :] = row i). The int16
        # index for position i lives at [i % 16, i // 16] of a [16, N/16]
        # block, tiled across the 128 partitions (8 gpsimd core stripes).
        li_pos = (
            li.reshape(NCHUNK, P, K).transpose(0, 2, 1).reshape(NCHUNK * K * P)
        )  # ordered by i = (c, k, p)
        wrap = li_pos.reshape(-1, 16).T.astype(np.int16)  # [16, N/16]
        gidx = np.ascontiguousarray(np.tile(wrap, (8, 1)))  # [128, N/16]
        w = np.ascontiguousarray(
            flat_w[t0 : t0 + TC]
            .reshape(NCHUNK, P, K)
            .transpose(1, 0, 2)
            .reshape(P, NCHUNK * K)
            .astype(np.float32)
        )
        in_maps.append({"table": slab, "gidx": gidx, "wgt": w})
    return in_maps


_NC_CACHE = None


def run(
    hidden_states,
    expert_indices,
    expert_weights,
    expert_outputs,
    trace=False,
):
    global _NC_CACHE
    in_maps = _prepare_in_maps(expert_indices, expert_weights, expert_outputs)
    if _NC_CACHE is None:
        _NC_CACHE = _build()
    nc = _NC_CACHE
    res = run_bass_kernel_spmd(nc, in_maps, list(range(N_CORES)), trace=trace)
    outs = []
    for i in range(N_CORES):
        r = np.asarray(res.results[i]["out"])  # [P, NCHUNK*H] partition-major
        r = (
            r.reshape(P, NCHUNK, H)
            .transpose(1, 0, 2)
            .reshape(TC, H)
            .astype(np.float32)
        )
        outs.append(r)
    full = np.concatenate(outs, axis=0).reshape(B, S, H)
    return full, res


def kernel(hidden_states, expert_indices, expert_weights, expert_outputs):
    full, _ = run(hidden_states, expert_indices, expert_weights, expert_outputs)
    return full


# revision 32
# speedup vs baseline: 1.3302x; 1.3302x over previous
"""MoE expert-combine kernel for Trainium2 (raw Bass, hand-scheduled), 8-core SPMD.

Problem: out[b,s,:] = sum_k expert_weights[b,s,k] * expert_outputs[expert_indices[b,s,k], b, s, :]
  B,S,H = 4,2048,1024 ; E=8 ; K=2  (hidden_states is unused by the reference)

Sharding: flatten tokens t = b*S+s (8192 total); each of the 8 cores owns a
contiguous block of 1024 tokens. Each core receives the expert-output stack
sliced to its tokens and downcast to bf16 ([E, 1024, H] viewed as a row table
[E*1024, H]) plus host-precomputed gather row indices and f32 gate weights.
The output is written bf16 (partition-major) and upcast/reordered to f32 on
the host; the combined quantization error is ~2.5e-3 rel, far inside the 2e-2
gate, and it halves the DMA traffic (12MB -> 6MB per core).

Device schedule, per 128-token chunk c (token = c*128 + p):
 - gather: 4 SWDGE dma_gather ops (mlp gpsimd library), one per chunk PAIR
   (512 rows of 2KB each), round-robin across the 4 SWDGE queues. One op is
   ~1.3us of Q7 descriptor writing, so 4 ops keep the Q7 off the critical
   path while the 4 rings transfer in parallel. Indices are int16 in the
   documented [16-partition wrap x replicated-across-cores] layout.
 - combine, split across two engines so neither is the bottleneck:
     Act:  acc[c%4] = w0 * g0      (Copy activation with per-partition scale)
     DVE:  ot[c] = (w1 * g1) + acc (scalar_tensor_tensor)
   acc is a 4-deep ring buffer; Act waits on sem_v for the anti-dependency
   before reusing a slot (standalone wait; the op's own wait slot is spent on
   the gather semaphore).
 - store: HWDGE writes chunk pairs as [128, 2048] bf16 to a partition-major
   DRAM layout ([P, NCHUNK*H]) so each store descriptor is a contiguous 4KB.
Hand-placed semaphores, at most one sync-wait per compute instruction (walrus
codegen limit), and no end-of-block drain/barrier (the sync engine's final
sem_st wait covers every data dependency; the NEFF's own per-engine completion
chain runs regardless).
"""

import sys
import numpy as np

for _p in ("/opt/trn_rl_repo", "/opt/pypackages"):
    if _p not in sys.path:
        sys.path.append(_p)

import ml_dtypes

from concourse import bass, mybir
from concourse.bass_utils import run_bass_kernel_spmd

B, S, H = 4, 2048, 1024
E, K = 8, 2
N_CORES = 8
T = B * S              # 8192 tokens total
TC = T // N_CORES      # 1024 tokens per core
P = 128                # SBUF partitions
NCHUNK = TC // P       # 8 chunks of 128 tokens per core

ACC_DEPTH = 4           # acc ring buffer depth

_f32 = mybir.dt.float32
_bf16 = mybir.dt.bfloat16
_i32 = mybir.dt.int32

_BF16 = ml_dtypes.bfloat16


def _build():
    nc = bass.Bass(target_bir_lowering=False, dynamic_dma_scratch_size=32768)

    # Preamble instructions exist already (emitted by Bass.__init__); snapshot
    # them so the strip below touches only these, never user instructions.
    _preamble_names = {
        ins.name for bb in nc.m.functions[0].blocks for ins in bb.instructions
    }

    table = nc.declare_dram_parameter("table", [E * TC, H], _bf16, isOutput=False)
    # gather row indices, int32, chunk-major: [p, c*K+k] = row for token
    # (c*128+p), slot k
    gidx = nc.declare_dram_parameter("gidx", [P, NCHUNK * K], _i32, isOutput=False)
    wgt = nc.declare_dram_parameter("wgt", [P, NCHUNK * K], _f32, isOutput=False)
    # partition-major output: row p holds tokens (c*128+p) for c = 0..NCHUNK-1
    out = nc.declare_dram_parameter("out", [P, NCHUNK * H], _bf16, isOutput=True)

    with (
        nc.semaphore("sem_idx") as sem_idx,
        nc.semaphore("sem_prep") as sem_prep,
        nc.semaphore("sem_w") as sem_w,
        nc.semaphore("sem_acc") as sem_acc,
        nc.semaphore("sem_v") as sem_v,
        nc.semaphore("sem_st") as sem_st,
        nc.sbuf_tensor("gidx_t", [P, NCHUNK * K], _i32) as gidx_t,
        nc.sbuf_tensor("w_t", [P, NCHUNK * K], _f32) as w_t,
        nc.sbuf_tensor("g_t", [P, NCHUNK * K, H], _bf16) as g_t,
        nc.sbuf_tensor("ot_t", [P, NCHUNK * H], _bf16) as ot_t,
        nc.sbuf_tensor("acc_t", [P, ACC_DEPTH, H], _bf16) as acc_t,
    ):
        gather_sems = [nc.alloc_semaphore(f"sem_g{i}") for i in range(NCHUNK)]

        def sync_body(sync: bass.BassEngine):
            sync.dma_start(out=gidx_t[:], in_=gidx[:]).then_inc(sem_idx, 16)
            sync.dma_start(out=w_t[:], in_=wgt[:]).then_inc(sem_w, 16)
            for j in range(NCHUNK // 2):
                # chunk pair (2j, 2j+1) ready after DVE op 2j+2; the DRAM side
                # is contiguous per partition -> 4KB store descriptors
                sync.wait_ge(sem_v, 2 * j + 2)
                sync.dma_start(
                    out=out[:, 2 * j * H : (2 * j + 2) * H],
                    in_=ot_t[:, 2 * j * H : (2 * j + 2) * H],
                ).then_inc(sem_st, 16)
            # Final wait: keeps every sem update inside the program (safe for
            # re-execution). Costs nothing — the runtime teardown's per-engine
            # DRAINs wait for DMA-queue quiescence anyway.
            sync.wait_ge(sem_st, 16 * (NCHUNK // 2))

        def gpsimd_body(gpsimd: bass.BassGpSimd):
            # Base-firmware indirect DMA (InstDMACopy/SWDGE mainline), one op
            # per (chunk, k): 128 descriptors each, ~1.45us of Q7 desc-gen per
            # op but IMMEDIATE ring firing — transfers overlap desc-gen, and
            # there is no mlp-library load (measured 4-9us, serial, variable)
            # on the critical path. Net: gen-paced ~23us pipeline, lower
            # expected time and far lower variance than dma_gather's
            # lib-load + batched-doorbell (writes THEN transfers) pipeline.
            gpsimd.wait_ge(sem_idx, 16)
            for c in range(NCHUNK):
                for k in range(K):
                    m = c * K + k
                    gpsimd.indirect_dma_start(
                        out=g_t[:, m, :],
                        out_offset=None,
                        in_=table[:],
                        in_offset=bass.IndirectOffsetOnAxis(
                            ap=gidx_t[:, m : m + 1], axis=0
                        ),
                        # k=1 signals the whole chunk: the single SWDGE queue
                        # completes ops in FIFO order, so op 2c+1 done implies
                        # op 2c done. Act waits this one sem; DVE orders
                        # behind Act via sem_acc.
                    ).then_inc(gather_sems[c] if k == 1 else sem_prep, 16)

        def scalar_body(scalar: bass.BassEngine):
            # acc[c % ACC_DEPTH] = w0 * g0 on the Activation engine
            scalar.wait_ge(sem_w, 16)
            for c in range(NCHUNK):
                if c >= ACC_DEPTH:
                    # anti-dependency: DVE must have consumed acc slot c-4
                    scalar.wait_ge(sem_v, c - ACC_DEPTH + 1)
                m0 = c * K
                scalar.mul(
                    acc_t[:, c % ACC_DEPTH, :],
                    g_t[:, m0, :],
                    w_t[:, m0 : m0 + 1],
                )._wait_ge(gather_sems[c], 16).then_inc(sem_acc, 1)

        def vector_body(vector: bass.BassEngine):
            # ot[c] = (w1 * g1) + acc[c % ACC_DEPTH] ; the acc sem implies the
            # chunk's gather completed (Act waited on it), so one wait suffices
            vector.wait_ge(sem_w, 16)
            for c in range(NCHUNK):
                m1 = c * K + 1
                vector.scalar_tensor_tensor(
                    out=ot_t[:, c * H : (c + 1) * H],
                    in0=g_t[:, m1, :],
                    scalar=w_t[:, m1 : m1 + 1],
                    in1=acc_t[:, c % ACC_DEPTH, :],
                    op0=mybir.AluOpType.mult,
                    op1=mybir.AluOpType.add,
                )._wait_ge(sem_acc, c + 1).then_inc(sem_v, 1)

        # Emit every engine's stream directly into the entry basic block: no
        # per-engine body blocks means no branches, so the sequencers never
        # stall on an IRAM block fetch (~2.5us observed), and there is no
        # end-of-block drain/barrier either.
        sync_body(nc.sync)
        gpsimd_body(nc.gpsimd)
        scalar_body(nc.scalar)
        vector_body(nc.vector)

    # Strip the preamble's const-tile memsets and the post-init all-engine
    # barrier (~2.5us): this kernel never reads the const APs, and each
    # engine's register init precedes its user code in program order anyway.
    entry = nc.m.functions[0].blocks[0]
    drop = {
        ins.name
        for ins in entry.instructions
        if ins.name in _preamble_names
        and type(ins).__name__
        in ("InstMemset", "InstDrain", "InstEventSemaphore", "InstRegisterMove")
    }
    kept = [ins for ins in entry.instructions if ins.name not in drop]
    del entry.instructions[:]
    for ins in kept:
        entry.instructions.append(ins)

    # Lower InstISA pseudo-instructions (the mlp-library reload) to real ISA
    # bytes; raw walrus codegen rejects unlowered pseudos.
    mybir.codegen_inst_isa_subclasses(nc)

    nc.finalize()
    return nc


def _prepare_in_maps(expert_indices, expert_weights, expert_outputs):
    eo = np.asarray(expert_outputs, dtype=np.float32).reshape(E, T, H)
    eo16 = eo.astype(_BF16)
    flat_idx = np.asarray(expert_indices).reshape(T, K).astype(np.int32)
    flat_w = np.asarray(expert_weights, dtype=np.float32).reshape(T, K)
    t_local = np.arange(TC, dtype=np.int32)[:, None]
    in_maps = []
    for i in range(N_CORES):
        t0 = i * TC
        slab = np.ascontiguousarray(eo16[:, t0 : t0 + TC, :]).reshape(E * TC, H)
        li = flat_idx[t0 : t0 + TC] * TC + t_local  # [TC, K] row idx into slab
        # chunk-major: partition p of chunk c holds token c*128+p
        gidx = np.ascontiguousarray(
            li.reshape(NCHUNK, P, K).transpose(1, 0, 2).reshape(P, NCHUNK * K)
        )
        w = np.ascontiguousarray(
            flat_w[t0 : t0 + TC]
            .reshape(NCHUNK, P, K)
            .transpose(1, 0, 2)
            .reshape(P, NCHUNK * K)
            .astype(np.float32)
        )
        in_maps.append({"table": slab, "gidx": gidx, "wgt": w})
    return in_maps


_NC_CACHE = None


def run(
    hidden_states,
    expert_indices,
    expert_weights,
    expert_outputs,
    trace=False,
):
    global _NC_CACHE
    in_maps = _prepare_in_maps(expert_indices, expert_weights, expert_outputs)
    if _NC_CACHE is None:
        _NC_CACHE = _build()
    nc = _NC_CACHE
    res = run_bass_kernel_spmd(nc, in_maps, list(range(N_CORES)), trace=trace)
    outs = []
    for i in range(N_CORES):
        r = np.asarray(res.results[i]["out"])  # [P, NCHUNK*H] partition-major
        r = (
            r.reshape(P, NCHUNK, H)
            .transpose(1, 0, 2)
            .reshape(TC, H)
            .astype(np.float32)
        )
        outs.append(r)
    full = np.concatenate(outs, axis=0).reshape(B, S, H)
    return full, res


def kernel(hidden_states, expert_indices, expert_weights, expert_outputs):
    full, _ = run(hidden_states, expert_indices, expert_weights, expert_outputs)
    return full
